# revision 1
# baseline (speedup 1.0000x reference)
"""Multi-head attention with random-synthesizer blend + mask, on 8 Trainium2
NeuronCores.  v3: projections interleaved into the attention stream, ones-
column softmax sums, PE-broadcast normalization.

Sharding: data-parallel over batch (B=8 -> one batch element per core).

Per-core layouts ([partition, free]):
  - xT (q/k/v): [D, S] fp16, transposed+cast on host; maskT [S(k), S(q)] fp16;
    esynT[h] = exp((1-alpha)*syn[h].T) fp16 (host); c1 folded into Wq/bq.
  - qT/kT: [d_out, s] fp16. v_sb: [s, H*65] fp16 - per head 64 v-dims plus
    one all-ones column, so each PV matmul row 64 yields the softmax sum.
  - Attention per (h,kc): scores_T -> one [128,1024] exp (ACT) -> esynT and
    maskT multiplies (DVE, in place) -> PV accumulate into pav[0:65].
  - Q/K projection chunk hp+1 is emitted between head pairs so the PE stream
    stays dense while ACT/DVE chew on exp/multiplies.
  - Normalization: reciprocal_approx_fast on compact [2,1024] sums, cast to
    fp16, rank-1 ones-matmuls broadcast it into PSUM, one DVE multiply.
  - o-proj with host boeff = bv @ Wo + bo; output stored fp16.
"""

import math
import sys

sys.path.insert(0, "/opt/trn_rl_repo")

import numpy as np

import concourse.tile as tile
import concourse.mybir as mybir
from concourse import bacc
from concourse.bass_utils import run_bass_kernel_spmd

B, S, D, H = 8, 1024, 1024, 16
HD = D // H  # 64
N_CORES = 8
P = 128
SC = S // P  # 8
DC = D // P  # 8
NQ = 512
VW = HD + 1  # 65: v block width incl ones column

f32 = mybir.dt.float32
fp16 = mybir.dt.float16
AF = mybir.ActivationFunctionType
OP = mybir.AluOpType

TRACE = False
TRACE_TMPDIR = None
LAST_RESULTS = None

_CACHE = {}


def _emit(nc, tc, dram):
    w_d = {"q": dram["wq"], "k": dram["wk"], "v": dram["wv"], "o": dram["wo"]}
    x_d = {"q": dram["xq"], "k": dram["xk"], "v": dram["xv"]}
    out_d = dram["out"]

    with (
        tc.tile_pool(name="pers", bufs=1) as pers,
        tc.tile_pool(name="psmm", bufs=1, space="PSUM") as psmm,
        tc.tile_pool(name="psav", bufs=1, space="PSUM") as psav,
    ):
        # ---- constants ---------------------------------------------------
        ones_h = pers.tile([33, P], fp16, tag="ones_h")
        nc.vector.memset(ones_h[:], 1.0)
        bqk_sb = {}
        for nm in ("q", "k"):
            t = pers.tile([P, DC], f32, tag=f"b{nm}", name=f"b{nm}")
            nc.gpsimd.dma_start(out=t[:], in_=dram["b" + nm].rearrange("(c p) -> p c", p=P))
            bqk_sb[nm] = t
        bo_sb = pers.tile([1, D], fp16, tag="bo_sb")
        nc.gpsimd.dma_start(out=bo_sb[:], in_=dram["boeff"][None, :])

        # ---- persistent activations --------------------------------------
        qT = [pers.tile([P, S], fp16, tag=f"qT{i}", name=f"qT{i}") for i in range(DC)]
        kT = [pers.tile([P, S], fp16, tag=f"kT{i}", name=f"kT{i}") for i in range(DC)]
        v_sb = [pers.tile([P, H * VW], fp16, tag=f"v{i}", name=f"v{i}")
                for i in range(SC)]
        maskT = [pers.tile([P, S], fp16, tag=f"mT{i}", name=f"mT{i}")
                 for i in range(SC)]
        otn = [pers.tile([P, S], fp16, tag=f"otn{i}", name=f"otn{i}")
               for i in range(DC)]

        for kb in range(SC):
            nc.gpsimd.dma_start(out=maskT[kb][:], in_=dram["mskT"][kb * P:(kb + 1) * P, :])

        def load_tiles(pool, dsrc, prefix, bufs=1, eng=None):
            eng = eng or nc.sync
            tiles = []
            for ci in range(DC):
                t = pool.tile([P, D], fp16, tag=f"{prefix}{ci}", bufs=bufs,
                              name=f"{prefix}{ci}")
                eng.dma_start(out=t[:], in_=dsrc[ci * P:(ci + 1) * P, :])
                tiles.append(t)
            return tiles

        def load_w_chunk(pool, nm, do):
            # one contiguous DMA: host packs chunk do as [128, 8*128]
            t = pool.tile([P, D], fp16, tag=f"w{nm}c", bufs=2,
                          name=f"w{nm}{do}")
            nc.sync.dma_start(out=t[:], in_=w_d[nm][do])
            return [t[:, di * P:(di + 1) * P] for di in range(DC)]

        def qk_proj_chunk(nm, wct, xt, dst, do):
            ps = psmm.tile([P, S], f32, tag="mm", bufs=2, name=f"ps{nm}{do}")
            for sq in range(2):
                for di in range(DC):
                    nc.tensor.matmul(
                        ps[:, sq * NQ:(sq + 1) * NQ],
                        wct[di],
                        xt[di][:, sq * NQ:(sq + 1) * NQ],
                        start=(di == 0),
                        stop=(di == DC - 1),
                    )
            nc.scalar.activation(
                out=dst[do][:], in_=ps[:], func=AF.Identity,
                bias=bqk_sb[nm][:, do:do + 1],
            )

        def v_proj_chunk(pool, wt, sc):
            nc.gpsimd.memset(v_sb[sc][:], 1.0)
            xt = pool.tile([P, D], fp16, tag="xvc", bufs=2, name=f"xv{sc}")
            nc.sync.dma_start(out=xt[:], in_=x_d["v"][sc])
            xct = [xt[:, di * P:(di + 1) * P] for di in range(DC)]
            ps = psmm.tile([P, S], f32, tag="mm", bufs=2, name=f"psv{sc}")
            for dq in range(2):
                for di in range(DC):
                    nc.tensor.matmul(
                        ps[:, dq * NQ:(dq + 1) * NQ],
                        xct[di],
                        wt[di][:, dq * NQ:(dq + 1) * NQ],
                        start=(di == 0),
                        stop=(di == DC - 1),
                    )
            src = ps[:].rearrange("p (a r) -> p a r", r=HD)
            dst = v_sb[sc][:].rearrange("p (a r) -> p a r", r=VW)
            nc.scalar.copy(out=dst[:, :, 0:HD], in_=src[:, :, :])

        def head(h, ap, spair, vwork=None, filler=None):
            hp, hodd = h // 2, h % 2
            pav = psav.tile([P, S], f32, tag="av", bufs=1, name=f"pav{h}")
            for kc in range(SC):
                if vwork is not None:
                    vwork(kc)
                if filler is not None:
                    filler()
                syn_t = ap.tile([P, S], fp16, tag="synT", bufs=3,
                                name=f"sy{h}_{kc}")
                eng = nc.sync if kc % 2 == 0 else nc.gpsimd
                eng.dma_start(
                    out=syn_t[:], in_=dram["esyn"][h, kc * P:(kc + 1) * P, :]
                )
                ps = psmm.tile([P, S], f32, tag="mm", bufs=2, name="pss")
                for sq in range(2):
                    nc.tensor.matmul(
                        ps[:, sq * NQ:(sq + 1) * NQ],
                        kT[hp][hodd * HD:(hodd + 1) * HD, kc * P:(kc + 1) * P],
                        qT[hp][hodd * HD:(hodd + 1) * HD, sq * NQ:(sq + 1) * NQ],
                        start=True, stop=True,
                    )
                p = ap.tile([P, S], fp16, tag="p", bufs=3, name="p")
                nc.scalar.activation(out=p[:], in_=ps[:], func=AF.Exp)
                nc.vector.tensor_tensor(out=p[:], in0=p[:], in1=syn_t[:], op=OP.mult)
                nc.vector.tensor_tensor(out=p[:], in0=p[:], in1=maskT[kc][:], op=OP.mult)
                for sq in range(2):
                    nc.tensor.matmul(
                        pav[0:VW, sq * NQ:(sq + 1) * NQ],
                        v_sb[kc][:, h * VW:(h + 1) * VW],
                        p[:, sq * NQ:(sq + 1) * NQ],
                        start=(kc == 0), stop=(kc == SC - 1),
                    )
            # evacuate raw output (ACT) + softmax sums row (DVE)
            nc.scalar.copy(out=otn[hp][hodd * HD:(hodd + 1) * HD, :],
                           in_=pav[0:HD, :])
            nc.vector.tensor_copy(out=spair[32 * hodd:32 * hodd + 1, :],
                                  in_=pav[HD:VW, :])

        def norm(hp, ap, spair):
            # otn[hp] rows 0:64 = head 2hp, 64:128 = head 2hp+1
            # spair rows 1..31 hold 1.0 so the full-tile ops stay finite
            rec = ap.tile([33, S], f32, tag="rec", bufs=1, name=f"rc{hp}")
            nc.vector.reciprocal_approx_fast(out=rec[:], in_=spair[:])
            r16 = ap.tile([33, S], fp16, tag="rec16", bufs=1, name=f"rh{hp}")
            nc.gpsimd.tensor_copy(out=r16[:], in_=rec[:])
            rec16 = [r16[0:1, :], r16[32:33, :]]
            prec = psmm.tile([P, S], f32, tag="pmm", bufs=1, name=f"prc{hp}")
            for r in range(2):
                for sq in range(2):
                    nc.tensor.matmul(
                        prec[r * HD:(r + 1) * HD, sq * NQ:(sq + 1) * NQ],
                        ones_h[32 * r:32 * r + 1, 0:HD],
                        rec16[r][:, sq * NQ:(sq + 1) * NQ],
                        start=True, stop=True,
                    )
            nc.vector.tensor_tensor(out=otn[hp][:], in0=otn[hp][:],
                                    in1=prec[:], op=OP.mult)

        # ================= emission ======================================
        with (
            tc.tile_pool(name="projp", bufs=1) as projp,
            tc.tile_pool(name="attn", bufs=1) as ap,
        ):
            wcq0 = load_w_chunk(projp, "q", 0)
            xq_t = load_tiles(projp, x_d["q"], "xq")
            wck0 = load_w_chunk(projp, "k", 0)
            xk_t = load_tiles(projp, x_d["k"], "xk")
            wv_t = load_tiles(projp, w_d["v"], "wv", eng=nc.gpsimd)

            qk_proj_chunk("q", wcq0, xq_t, qT, 0)
            qk_proj_chunk("k", wck0, xk_t, kT, 0)

            # remaining q/k projection chunks drained 2-3 matmuls per kc
            # inside the attention loops: keeps the PE duty cycle high so
            # the HAM clock gate stays at full rate.
            proj_work = []
            for do in range(1, DC):
                for nm, xt, dst in (("q", xq_t, qT), ("k", xk_t, kT)):
                    def mk_chunk(nm=nm, xt=xt, dst=dst, do=do):
                        state = {}

                        def start():
                            state["w"] = load_w_chunk(projp, nm, do)
                            state["ps"] = psmm.tile(
                                [P, S], f32, tag="pmm", bufs=1,
                                name=f"pp{nm}{do}")

                        def mm(sq, di):
                            nc.tensor.matmul(
                                state["ps"][:, sq * NQ:(sq + 1) * NQ],
                                state["w"][di],
                                xt[di][:, sq * NQ:(sq + 1) * NQ],
                                start=(di == 0),
                                stop=(di == DC - 1),
                            )

                        def evac():
                            nc.scalar.activation(
                                out=dst[do][:], in_=state["ps"][:],
                                func=AF.Identity,
                                bias=bqk_sb[nm][:, do:do + 1],
                            )

                        ops = [start]
                        ops += [lambda sq=sq, di=di: mm(sq, di)
                                for sq in range(2) for di in range(DC)]
                        ops.append(evac)
                        return ops
                    proj_work.extend(mk_chunk())
            proj_work.reverse()  # pop() from the front

            # pair hp+1 needs chunks q/k (hp+1) complete: 2 chunk-units
            # (2*18 ops) per pair, drained over 16 kc slots -> >=3/slot
            def drain(n):
                def f():
                    for _ in range(n):
                        if proj_work:
                            proj_work.pop()()
                return f

            wt_o = None
            spairs = {}

            for hp in range(DC):
                if hp == 1:
                    wt_o = load_tiles(projp, w_d["o"], "wo")
                spair = ap.tile([33, S], f32, tag="spair", bufs=2,
                                name=f"sp{hp}")
                spairs[hp] = spair
                nc.gpsimd.memset(spair[:], 1.0)
                if hp == 0:
                    head(0, ap, spair,
                         vwork=lambda kc: v_proj_chunk(projp, wv_t, kc),
                         filler=drain(2))
                else:
                    # chunks q/k(hp) must be fully emitted before this pair
                    while len(proj_work) > 18 * 2 * (DC - 1 - hp):
                        proj_work.pop()()
                    head(2 * hp, ap, spair, filler=drain(3))
                if hp > 0:
                    # deferred: previous pair's normalization hides behind
                    # this pair's attention stream
                    norm(hp - 1, ap, spairs.pop(hp - 1))
                head(2 * hp + 1, ap, spair, filler=drain(3))
            while proj_work:
                proj_work.pop()()
            norm(DC - 1, ap, spairs.pop(DC - 1))

            # ============= output projection =============================
            for sc in range(SC):
                ps = psmm.tile([P, S], f32, tag="mm", bufs=2, name=f"pso{sc}")
                for dq in range(2):
                    for ci in range(DC):
                        nc.tensor.matmul(
                            ps[:, dq * NQ:(dq + 1) * NQ],
                            otn[ci][:, sc * P:(sc + 1) * P],
                            wt_o[ci][:, dq * NQ:(dq + 1) * NQ],
                            start=(ci == 0), stop=False,
                        )
                    nc.tensor.matmul(
                        ps[:, dq * NQ:(dq + 1) * NQ], ones_h[0:1, :P],
                        bo_sb[:, dq * NQ:(dq + 1) * NQ],
                        start=False, stop=True,
                    )
                osb = ap.tile([P, S], fp16, tag="osb", bufs=2, name="osb")
                nc.scalar.copy(out=osb[:], in_=ps[:])
                nc.sync.dma_start(out=out_d[sc * P:(sc + 1) * P, :], in_=osb[:])


def _build():
    nc = bacc.Bacc("TRN2", debug=False)
    dram = {
        "xq": nc.declare_dram_parameter("xq", [D, S], fp16, isOutput=False),
        "xk": nc.declare_dram_parameter("xk", [D, S], fp16, isOutput=False),
        "xv": nc.declare_dram_parameter("xv", [SC, P, D], fp16, isOutput=False),
        "mskT": nc.declare_dram_parameter("mskT", [S, S], fp16, isOutput=False),
        "wq": nc.declare_dram_parameter("wq", [DC, P, D], fp16, isOutput=False),
        "wk": nc.declare_dram_parameter("wk", [DC, P, D], fp16, isOutput=False),
        "wv": nc.declare_dram_parameter("wv", [D, D], fp16, isOutput=False),
        "wo": nc.declare_dram_parameter("wo", [D, D], fp16, isOutput=False),
        "bq": nc.declare_dram_parameter("bq", [D], f32, isOutput=False),
        "bk": nc.declare_dram_parameter("bk", [D], f32, isOutput=False),
        "boeff": nc.declare_dram_parameter("boeff", [D], fp16, isOutput=False),
        "esyn": nc.declare_dram_parameter("esyn", [H, S, S], fp16, isOutput=False),
        "out": nc.declare_dram_parameter("out", [S, D], fp16, isOutput=True),
    }
    with tile.TileContext(nc) as tc:
        _emit(nc, tc, dram)
    nc.compile()
    return nc


def _prep(inputs):
    q = np.asarray(inputs["query"], np.float32)
    k = np.asarray(inputs["key"], np.float32)
    v = np.asarray(inputs["value"], np.float32)
    msk = np.asarray(inputs["mask"], np.int32)
    ws = {nm: np.asarray(inputs["W" + nm], np.float32) for nm in "qkvo"}
    bs = {nm: np.asarray(inputs["b" + nm], np.float32) for nm in "qkvo"}
    alpha = 1.0 / (1.0 + math.exp(-float(np.asarray(inputs["alpha_param"]).ravel()[0])))
    c1 = alpha / math.sqrt(HD)
    c2 = 1.0 - alpha

    esyn = np.exp(
        c2 * np.asarray(inputs["syn_scores"])[:, :S, :S].transpose(0, 2, 1)
    ).astype(np.float16)
    esyn = np.ascontiguousarray(esyn)
    boeff = (bs["v"].astype(np.float64) @ ws["o"].astype(np.float64)
             + bs["o"]).astype(np.float16)

    def chunk_pack(w):
        # [do, p, di*P + c] = w[di*P + p, do*P + c]
        w4 = w.reshape(DC, P, DC, P)          # [di, p, do, c]
        return np.ascontiguousarray(
            w4.transpose(2, 1, 0, 3).reshape(DC, P, D))

    common = {
        "wq": chunk_pack((c1 * ws["q"]).astype(np.float16)),
        "wk": chunk_pack(ws["k"].astype(np.float16)),
        "wv": ws["v"].astype(np.float16),
        "wo": ws["o"].astype(np.float16),
        "bq": (c1 * bs["q"]).astype(np.float32),
        "bk": bs["k"],
        "boeff": boeff,
        "esyn": esyn,
    }
    in_maps = []
    for b in range(B):
        m = dict(common)
        m["xq"] = np.ascontiguousarray(q[b].T.astype(np.float16))
        m["xk"] = np.ascontiguousarray(k[b].T.astype(np.float16))
        m["xv"] = chunk_pack(v[b].T.astype(np.float16))
        m["mskT"] = np.ascontiguousarray(msk[b].T.astype(np.float16))
        in_maps.append(m)
    return in_maps


def kernel(**inputs):
    global LAST_RESULTS
    if "nc" not in _CACHE:
        _CACHE["nc"] = _build()
    nc = _CACHE["nc"]
    in_maps = _prep(inputs)

    kwargs = {}
    if TRACE:
        kwargs["trace"] = True
        if TRACE_TMPDIR:
            kwargs["tmpdir"] = TRACE_TMPDIR
    res = run_bass_kernel_spmd(nc, in_maps, core_ids=list(range(N_CORES)), **kwargs)
    LAST_RESULTS = res
    return np.stack(
        [res.results[b]["out"].astype(np.float32) for b in range(B)], axis=0
    )



# revision 4
# speedup vs baseline: 1.4240x; 1.4240x over previous
"""Multi-head attention with random-synthesizer blend + mask, on 8 Trainium2
NeuronCores.  v4: host-fused esyn*mask (one DVE multiply per score tile),
PV lagged one key-chunk behind QK so the PE never stalls on the exp chain,
transposed output projection with per-partition bias, DVE-based norm cast.

Sharding: data-parallel over batch (B=8 -> one batch element per core).

Per-core layouts ([partition, free]):
  - xT (q/k/v): [D, S] fp16, transposed+cast on host; c1 folded into Wq/bq.
  - emsk[h,kc] = (exp((1-alpha)*syn[h].T) * mask.T) fp16 tiles, host-fused.
  - qT/kT: [d_out, s] fp16. v_sb: [s, H*65] fp16 - per head 64 v-dims plus
    one all-ones column, so each PV matmul row 64 yields the softmax sum.
  - Attention per (h,kc): scores_T -> exp (ACT) -> one emsk multiply (DVE)
    -> PV accumulate into pav[0:65].  PV for chunk kc-1 is emitted after
    the projection-fill ops of chunk kc, so the PE stream never waits.
  - Q/K projection chunk hp+1 drained between head pairs (PE stays dense).
  - Normalization: reciprocal_approx_fast on [33,1024] sums, DVE cast to
    fp16, rank-1 ones-matmuls broadcast into PSUM, one DVE multiply.
  - o-proj transposed (Wo chunks stationary, otn moving): out^T[do,s] with
    boeff = bv @ Wo + bo applied as per-partition ACT bias; host transposes
    back.  Output stored fp16 [D, S].
"""

import math
import sys

sys.path.insert(0, "/opt/trn_rl_repo")

import numpy as np

import concourse.tile as tile
import concourse.mybir as mybir
from concourse import bacc
from concourse.bass_utils import run_bass_kernel_spmd

B, S, D, H = 8, 1024, 1024, 16
HD = D // H  # 64
N_CORES = 8
P = 128
SC = S // P  # 8
DC = D // P  # 8
NQ = 512
VW = HD + 1  # 65: v block width incl ones column

f32 = mybir.dt.float32
fp16 = mybir.dt.float16
AF = mybir.ActivationFunctionType
OP = mybir.AluOpType

TRACE = False
TRACE_TMPDIR = None
LAST_RESULTS = None

_CACHE = {}


def _emit(nc, tc, dram):
    w_d = {"q": dram["wq"], "k": dram["wk"], "v": dram["wv"], "o": dram["wo"]}
    x_d = {"q": dram["xq"], "k": dram["xk"], "v": dram["xv"]}
    out_d = dram["out"]

    with (
        tc.tile_pool(name="pers", bufs=1) as pers,
        tc.tile_pool(name="psmm", bufs=1, space="PSUM") as psmm,
        tc.tile_pool(name="psav", bufs=1, space="PSUM") as psav,
    ):
        # ---- constants ---------------------------------------------------
        ones_h = pers.tile([33, P], fp16, tag="ones_h")
        nc.vector.memset(ones_h[:], 1.0)
        bqk_sb = {}
        for nm in ("q", "k"):
            t = pers.tile([P, DC], f32, tag=f"b{nm}", name=f"b{nm}")
            nc.gpsimd.dma_start(out=t[:], in_=dram["b" + nm].rearrange("(c p) -> p c", p=P))
            bqk_sb[nm] = t
        bo_sb = pers.tile([P, DC], f32, tag="bo_sb")
        nc.gpsimd.dma_start(out=bo_sb[:], in_=dram["boeff"].rearrange("(c p) -> p c", p=P))

        # ---- persistent activations --------------------------------------
        qT = [pers.tile([P, S], fp16, tag=f"qT{i}", name=f"qT{i}") for i in range(DC)]
        kT = [pers.tile([P, S], fp16, tag=f"kT{i}", name=f"kT{i}") for i in range(DC)]
        v_sb = [pers.tile([P, H * VW], fp16, tag=f"v{i}", name=f"v{i}")
                for i in range(SC)]
        otn = [pers.tile([P, S], fp16, tag=f"otn{i}", name=f"otn{i}")
               for i in range(DC)]

        def load_tiles(pool, dsrc, prefix, bufs=1, eng=None, chunked=False):
            eng = eng or nc.sync
            tiles = []
            for ci in range(DC):
                t = pool.tile([P, D], fp16, tag=f"{prefix}{ci}", bufs=bufs,
                              name=f"{prefix}{ci}")
                src = dsrc[ci] if chunked else dsrc[ci * P:(ci + 1) * P, :]
                eng.dma_start(out=t[:], in_=src)
                tiles.append(t)
            return tiles

        def load_w_chunk(pool, nm, do):
            # one contiguous DMA: host packs chunk do as [128, 8*128]
            t = pool.tile([P, D], fp16, tag=f"w{nm}c", bufs=2,
                          name=f"w{nm}{do}")
            nc.sync.dma_start(out=t[:], in_=w_d[nm][do])
            return [t[:, di * P:(di + 1) * P] for di in range(DC)]

        def qk_proj_chunk(nm, wct, xt, dst, do):
            ps = psmm.tile([P, S], f32, tag="mm", bufs=2, name=f"ps{nm}{do}")
            for sq in range(2):
                for di in range(DC):
                    nc.tensor.matmul(
                        ps[:, sq * NQ:(sq + 1) * NQ],
                        wct[di],
                        xt[di][:, sq * NQ:(sq + 1) * NQ],
                        start=(di == 0),
                        stop=(di == DC - 1),
                    )
            nc.scalar.activation(
                out=dst[do][:], in_=ps[:], func=AF.Identity,
                bias=bqk_sb[nm][:, do:do + 1],
            )

        def v_proj_chunk(pool, wt, sc):
            nc.gpsimd.memset(v_sb[sc][:], 1.0)
            xt = pool.tile([P, D], fp16, tag="xvc", bufs=2, name=f"xv{sc}")
            nc.sync.dma_start(out=xt[:], in_=x_d["v"][sc])
            xct = [xt[:, di * P:(di + 1) * P] for di in range(DC)]
            ps = psmm.tile([P, S], f32, tag="mm", bufs=2, name=f"psv{sc}")
            for dq in range(2):
                for di in range(DC):
                    nc.tensor.matmul(
                        ps[:, dq * NQ:(dq + 1) * NQ],
                        xct[di],
                        wt[di][:, dq * NQ:(dq + 1) * NQ],
                        start=(di == 0),
                        stop=(di == DC - 1),
                    )
            src = ps[:].rearrange("p (a r) -> p a r", r=HD)
            dst = v_sb[sc][:].rearrange("p (a r) -> p a r", r=VW)
            nc.scalar.copy(out=dst[:, :, 0:HD], in_=src[:, :, :])

        def head(h, ap, spair, vwork=None, filler=None):
            hp, hodd = h // 2, h % 2
            pav = psav.tile([P, S], f32, tag="av", bufs=1, name=f"pav{h}")
            pwork = [None] * SC  # p tiles pending PV

            def pv(kc):
                p = pwork[kc]
                for sq in range(2):
                    nc.tensor.matmul(
                        pav[0:VW, sq * NQ:(sq + 1) * NQ],
                        v_sb[kc][:, h * VW:(h + 1) * VW],
                        p[:, sq * NQ:(sq + 1) * NQ],
                        start=(kc == 0), stop=(kc == SC - 1),
                    )

            for kc in range(SC + 1):
                if kc < SC:
                    if vwork is not None:
                        vwork(kc)
                    emsk_t = ap.tile([P, S], fp16, tag="synT", bufs=4,
                                     name=f"em{h}_{kc}")
                    eng = nc.sync if kc % 2 == 0 else nc.gpsimd
                    eng.dma_start(
                        out=emsk_t[:], in_=dram["emsk"][h, kc * P:(kc + 1) * P, :]
                    )
                    ps = psmm.tile([P, S], f32, tag="mm", bufs=2, name="pss")
                    for sq in range(2):
                        nc.tensor.matmul(
                            ps[:, sq * NQ:(sq + 1) * NQ],
                            kT[hp][hodd * HD:(hodd + 1) * HD, kc * P:(kc + 1) * P],
                            qT[hp][hodd * HD:(hodd + 1) * HD, sq * NQ:(sq + 1) * NQ],
                            start=True, stop=True,
                        )
                    p = ap.tile([P, S], fp16, tag="p", bufs=3, name="p")
                    nc.scalar.activation(out=p[:], in_=ps[:], func=AF.Exp)
                    nc.vector.tensor_tensor(out=p[:], in0=p[:], in1=emsk_t[:],
                                            op=OP.mult)
                    pwork[kc] = p
                if filler is not None:
                    filler()
                if kc > 0:
                    pv(kc - 1)
            # evacuate raw output (ACT) + softmax sums row (DVE)
            nc.scalar.copy(out=otn[hp][hodd * HD:(hodd + 1) * HD, :],
                           in_=pav[0:HD, :])
            nc.vector.tensor_copy(out=spair[32 * hodd:32 * hodd + 1, :],
                                  in_=pav[HD:VW, :])

        def norm(hp, ap, spair):
            # otn[hp] rows 0:64 = head 2hp, 64:128 = head 2hp+1
            # spair rows 1..31 hold 1.0 so the full-tile ops stay finite
            rec = ap.tile([33, S], f32, tag="rec", bufs=1, name=f"rc{hp}")
            nc.vector.reciprocal_approx_fast(out=rec[:], in_=spair[:])
            r16 = ap.tile([33, S], fp16, tag="rec16", bufs=1, name=f"rh{hp}")
            nc.vector.tensor_copy(out=r16[:], in_=rec[:])
            rec16 = [r16[0:1, :], r16[32:33, :]]
            prec = psmm.tile([P, S], f32, tag="pmm", bufs=1, name=f"prc{hp}")
            for r in range(2):
                for sq in range(2):
                    nc.tensor.matmul(
                        prec[r * HD:(r + 1) * HD, sq * NQ:(sq + 1) * NQ],
                        ones_h[32 * r:32 * r + 1, 0:HD],
                        rec16[r][:, sq * NQ:(sq + 1) * NQ],
                        start=True, stop=True,
                    )
            nc.vector.tensor_tensor(out=otn[hp][:], in0=otn[hp][:],
                                    in1=prec[:], op=OP.mult)

        # ================= emission ======================================
        with (
            tc.tile_pool(name="projp", bufs=1) as projp,
            tc.tile_pool(name="attn", bufs=1) as ap,
        ):
            wcq0 = load_w_chunk(projp, "q", 0)
            xq_t = load_tiles(projp, x_d["q"], "xq")
            wck0 = load_w_chunk(projp, "k", 0)
            xk_t = load_tiles(projp, x_d["k"], "xk")
            wv_t = load_tiles(projp, w_d["v"], "wv", eng=nc.gpsimd)

            qk_proj_chunk("q", wcq0, xq_t, qT, 0)
            qk_proj_chunk("k", wck0, xk_t, kT, 0)

            # remaining q/k projection chunks drained 2-3 matmuls per kc
            # inside the attention loops: keeps the PE duty cycle high so
            # the HAM clock gate stays at full rate.
            proj_work = []
            for do in range(1, DC):
                for nm, xt, dst in (("q", xq_t, qT), ("k", xk_t, kT)):
                    def mk_chunk(nm=nm, xt=xt, dst=dst, do=do):
                        state = {}

                        def start():
                            state["w"] = load_w_chunk(projp, nm, do)
                            state["ps"] = psmm.tile(
                                [P, S], f32, tag="pmm", bufs=1,
                                name=f"pp{nm}{do}")

                        def mm(sq, di):
                            nc.tensor.matmul(
                                state["ps"][:, sq * NQ:(sq + 1) * NQ],
                                state["w"][di],
                                xt[di][:, sq * NQ:(sq + 1) * NQ],
                                start=(di == 0),
                                stop=(di == DC - 1),
                            )

                        def evac():
                            nc.scalar.activation(
                                out=dst[do][:], in_=state["ps"][:],
                                func=AF.Identity,
                                bias=bqk_sb[nm][:, do:do + 1],
                            )

                        ops = [start]
                        ops += [lambda sq=sq, di=di: mm(sq, di)
                                for sq in range(2) for di in range(DC)]
                        ops.append(evac)
                        return ops
                    proj_work.extend(mk_chunk())
            proj_work.reverse()  # pop() from the front

            # pair hp+1 needs chunks q/k (hp+1) complete: 2 chunk-units
            # (2*18 ops) per pair, drained over 16 kc slots -> >=3/slot
            def drain(n):
                def f():
                    for _ in range(n):
                        if proj_work:
                            proj_work.pop()()
                return f

            wt_o = None
            spairs = {}

            for hp in range(DC):
                if hp == 1:
                    wt_o = load_tiles(projp, w_d["o"], "wo", chunked=True)
                spair = ap.tile([33, S], f32, tag="spair", bufs=2,
                                name=f"sp{hp}")
                spairs[hp] = spair
                nc.gpsimd.memset(spair[:], 1.0)
                if hp == 0:
                    head(0, ap, spair,
                         vwork=lambda kc: v_proj_chunk(projp, wv_t, kc),
                         filler=drain(2))
                else:
                    # chunks q/k(hp) must be fully emitted before this pair
                    while len(proj_work) > 18 * 2 * (DC - 1 - hp):
                        proj_work.pop()()
                    head(2 * hp, ap, spair, filler=drain(3))
                if hp > 0:
                    # deferred: previous pair's normalization hides behind
                    # this pair's attention stream
                    norm(hp - 1, ap, spairs.pop(hp - 1))
                head(2 * hp + 1, ap, spair, filler=drain(3))
            while proj_work:
                proj_work.pop()()
            norm(DC - 1, ap, spairs.pop(DC - 1))

            # ============= output projection (transposed) ================
            # out^T[do*P+p, s] = sum_d Wo[d, do*P+p] * otn[d, s] + boeff
            for do in range(DC):
                ps = psmm.tile([P, S], f32, tag="mm", bufs=2, name=f"pso{do}")
                for sq in range(2):
                    for di in range(DC):
                        nc.tensor.matmul(
                            ps[:, sq * NQ:(sq + 1) * NQ],
                            wt_o[do][:, di * P:(di + 1) * P],
                            otn[di][:, sq * NQ:(sq + 1) * NQ],
                            start=(di == 0), stop=(di == DC - 1),
                        )
                osb = ap.tile([P, S], fp16, tag="osb", bufs=2, name="osb")
                nc.scalar.activation(
                    out=osb[:], in_=ps[:], func=AF.Identity,
                    bias=bo_sb[:, do:do + 1],
                )
                nc.sync.dma_start(out=out_d[do * P:(do + 1) * P, :], in_=osb[:])


def _build():
    nc = bacc.Bacc("TRN2", debug=False)
    dram = {
        "xq": nc.declare_dram_parameter("xq", [D, S], fp16, isOutput=False),
        "xk": nc.declare_dram_parameter("xk", [D, S], fp16, isOutput=False),
        "xv": nc.declare_dram_parameter("xv", [SC, P, D], fp16, isOutput=False),
        "wq": nc.declare_dram_parameter("wq", [DC, P, D], fp16, isOutput=False),
        "wk": nc.declare_dram_parameter("wk", [DC, P, D], fp16, isOutput=False),
        "wv": nc.declare_dram_parameter("wv", [D, D], fp16, isOutput=False),
        "wo": nc.declare_dram_parameter("wo", [DC, P, D], fp16, isOutput=False),
        "bq": nc.declare_dram_parameter("bq", [D], f32, isOutput=False),
        "bk": nc.declare_dram_parameter("bk", [D], f32, isOutput=False),
        "boeff": nc.declare_dram_parameter("boeff", [D], f32, isOutput=False),
        "emsk": nc.declare_dram_parameter("emsk", [H, S, S], fp16, isOutput=False),
        "out": nc.declare_dram_parameter("out", [D, S], fp16, isOutput=True),
    }
    with tile.TileContext(nc) as tc:
        _emit(nc, tc, dram)
    nc.compile()
    return nc


def _prep(inputs):
    q = np.asarray(inputs["query"], np.float32)
    k = np.asarray(inputs["key"], np.float32)
    v = np.asarray(inputs["value"], np.float32)
    msk = np.asarray(inputs["mask"], np.int32)
    ws = {nm: np.asarray(inputs["W" + nm], np.float32) for nm in "qkvo"}
    bs = {nm: np.asarray(inputs["b" + nm], np.float32) for nm in "qkvo"}
    alpha = 1.0 / (1.0 + math.exp(-float(np.asarray(inputs["alpha_param"]).ravel()[0])))
    c1 = alpha / math.sqrt(HD)
    c2 = 1.0 - alpha

    # esynT[h][k, q] = exp(c2 * syn[h][q, k]) in fp16
    esynT = np.exp(
        c2 * np.asarray(inputs["syn_scores"], np.float32)[:, :S, :S].transpose(0, 2, 1)
    ).astype(np.float16)
    boeff = (bs["v"].astype(np.float64) @ ws["o"].astype(np.float64)
             + bs["o"]).astype(np.float32)

    def chunk_pack(w):
        # [do, p, di*P + c] = w[di*P + p, do*P + c]
        w4 = w.reshape(DC, P, DC, P)          # [di, p, do, c]
        return np.ascontiguousarray(
            w4.transpose(2, 1, 0, 3).reshape(DC, P, D))

    common = {
        "wq": chunk_pack((c1 * ws["q"]).astype(np.float16)),
        "wk": chunk_pack(ws["k"].astype(np.float16)),
        "wv": ws["v"].astype(np.float16),
        "wo": chunk_pack(ws["o"].astype(np.float16)),
        "bq": (c1 * bs["q"]).astype(np.float32),
        "bk": bs["k"],
        "boeff": boeff,
    }
    in_maps = []
    for b in range(B):
        m = dict(common)
        m["xq"] = np.ascontiguousarray(q[b].T.astype(np.float16))
        m["xk"] = np.ascontiguousarray(k[b].T.astype(np.float16))
        m["xv"] = chunk_pack(v[b].T.astype(np.float16))
        # emsk[h][k, q] = esynT[h][k, q] * mask[b][q, k]
        mTb = np.ascontiguousarray(msk[b].T).astype(np.float16)
        m["emsk"] = esynT * mTb[None, :, :]
        in_maps.append(m)
    return in_maps


def kernel(**inputs):
    global LAST_RESULTS
    if "nc" not in _CACHE:
        _CACHE["nc"] = _build()
    nc = _CACHE["nc"]
    in_maps = _prep(inputs)

    kwargs = {}
    if TRACE:
        kwargs["trace"] = True
        if TRACE_TMPDIR:
            kwargs["tmpdir"] = TRACE_TMPDIR
    res = run_bass_kernel_spmd(nc, in_maps, core_ids=list(range(N_CORES)), **kwargs)
    LAST_RESULTS = res
    return np.stack(
        [res.results[b]["out"].astype(np.float32).T for b in range(B)], axis=0
    )
